# revision 9
# baseline (speedup 1.0000x reference)
"""Trainium2 Bass kernel for nn_AutoregressiveGRUWithAttention.

Strategy (data-parallel over batch, 8 cores x 128 batch):
  Feature-on-partition layout, bf16 state, tanh-only gates
  (sigmoid(v)=0.5+0.5tanh(v/2) folded into weights; z negated).

  v2 changes vs baseline:
  - Doubled hidden state hd = 2h (h-rows of all h-consuming weight blocks
    pre-scaled by 0.5). GRU mix tail becomes:
      rr  = (1+tz) * n            (DVE, on chain)
      w2  = (tz-1) * hd_prev      (GPSIMD, off chain)
      hd' = rr - 0.5*w2           (DVE)  == 2*((1-z)n + z h)
    replacing the baseline's p1/pp/rr/h' 4-op tail.
  - y outputs accumulate in dedicated PSUM banks (32 steps per bank,
    ping-pong) and are evacuated by one ScalarE copy per 32 steps instead
    of a per-step scalar.copy.
  - Attention tail is its own software pipeline: e=exp(logits) bf16,
    eo=e*o (GPSIMD bf16), separate s/acc accumulators so rec=1/s does not
    wait for the eo add; attn = acc*rec (DVE).
  - State tiles (hd / attn / oB) are manually ping-ponged to remove
    write-after-read hazards from the recurrence chain.

PSUM banks: pr, pz, pa, pb (bufs=1 each), ly logits (bufs=2),
            y-accum (bufs=2, [128, 32*13]).
"""
import numpy as np
import ml_dtypes

B, L, T, IN, H, OUT = 1024, 64, 128, 13, 64, 13
NCORES, BL = 8, 128
BIG = 60.0
BF16 = ml_dtypes.bfloat16
YGRP = 32                      # y slots per psum bank

# wh pack column offsets [65 x 653] (bf16)
_HR, _HZ, _HB, _CR, _CZ, _CA, _WA = 0, 64, 128, 192, 256, 320, 384
_FR, _FZ, _FA2, _WF = 448, 512, 576, 640
_WH_COLS = 653
# wx pack column offsets [14 x 192] (bf16)
_XR, _XZ, _XA = 0, 64, 128
_WX_COLS = 192

LAST_EXEC_NS = None
TRACE = False
TRACE_DIR = None
WARM_DUMMIES = 0


def _prep_weights(Wih, Whh, bih, bhh, Wf, bf, Wa, ba):
    f8 = np.float64
    Wih, Whh, bih, bhh, Wf, bf, Wa, ba = [np.asarray(a, f8) for a in
                                          (Wih, Whh, bih, bhh, Wf, bf, Wa, ba)]
    Wr, Wz, Wn = Wih[0:H], Wih[H:2 * H], Wih[2 * H:3 * H]
    Ur, Uz, Un = Whh[0:H], Whh[H:2 * H], Whh[2 * H:3 * H]
    br_i, bz_i, bn_i = bih[0:H], bih[H:2 * H], bih[2 * H:3 * H]
    br_h, bz_h, bn_h = bhh[0:H], bhh[H:2 * H], bhh[2 * H:3 * H]

    def blk(rows, rowbias, scale):
        m = np.zeros((H + 1, rows.shape[0]), f8)
        m[0:H] = scale * rows.T
        m[H] = scale * rowbias
        return m

    def half(m):
        # doubled-h: blocks that multiply hd get their h-rows halved
        m2 = m.copy()
        m2[0:H] *= 0.5
        return m2

    HRm = blk(Ur, br_i + br_h, 0.5)
    HZm = blk(Uz, bz_i + bz_h, -0.5)
    HBm = blk(Un, bn_h, 0.5)                        # B' = 0.5*(nh + bhh_n)
    CRm = blk((Wf.T @ Wr.T).T, bf @ Wr.T, 0.5)
    CZm = blk((Wf.T @ Wz.T).T, bf @ Wz.T, -0.5)
    CAm = blk((Wf.T @ Wn.T).T, bf @ Wn.T + bn_i, 1.0)
    WAm = blk(Wa, ba, 1.0)

    wh = np.zeros((H + 1, _WH_COLS), f8)
    for col, m in ((_HR, half(HRm)), (_HZ, half(HZm)), (_HB, half(HBm)),
                   (_CR, CRm), (_CZ, CZm), (_CA, CAm), (_WA, WAm),
                   (_FR, half(HRm + CRm)), (_FZ, half(HZm + CZm)),
                   (_FA2, half(CAm + HBm))):
        wh[:, col:col + H] = m
    wh[0:H, _WF:_WF + OUT] = Wf.T
    wh[H, _WF:_WF + OUT] = bf

    wx = np.zeros((IN + 1, _WX_COLS), f8)
    wx[0:IN, _XR:_XR + H] = 0.5 * Wr.T
    wx[0:IN, _XZ:_XZ + H] = -0.5 * Wz.T
    wx[0:IN, _XA:_XA + H] = Wn.T
    wx[IN, _XA:_XA + H] = bn_i

    mrow = np.full((1, H), -0.5 * BIG, f8)
    return dict(
        wh=np.ascontiguousarray(wh, BF16),
        wx=np.ascontiguousarray(wx, BF16),
        mrow=np.ascontiguousarray(mrow, BF16),
    )


def _prep_core(x_core, len_core, l_steps=L):
    x_core = np.asarray(x_core, np.float32)
    xT = np.zeros((IN + 1, l_steps, BL), np.float32)
    xT[0:IN] = np.transpose(x_core[:, 0:l_steps, :], (2, 1, 0))
    xT[IN] = 1.0
    valid = (np.arange(l_steps)[:, None] < np.asarray(len_core)[None, :])
    invm = (1.0 - valid.astype(np.float32)).reshape(1, l_steps * BL)
    m63 = valid[l_steps - 1].astype(np.float32)
    m63bc = np.broadcast_to(m63, (H, BL)).astype(np.float32)
    m63b = np.concatenate([m63bc, 0.5 * m63bc], axis=1)   # [H, 2*BL]
    return (np.ascontiguousarray(xT.reshape(IN + 1, l_steps * BL), BF16),
            np.ascontiguousarray(invm, BF16),
            np.ascontiguousarray(m63b, np.float32))


def build_nc(l_steps=L, t_steps=T, compile=True, WARM=None):
    if WARM is None:
        WARM = WARM_DUMMIES
    import concourse.bacc as bacc
    import concourse.tile as tile
    from concourse import mybir
    from contextlib import ExitStack

    f32 = mybir.dt.float32
    bf = mybir.dt.bfloat16
    AF = mybir.ActivationFunctionType
    OP = mybir.AluOpType

    assert t_steps % YGRP == 0
    n_ygrp = t_steps // YGRP

    nc = bacc.Bacc("TRN2", target_bir_lowering=False, debug=False,
                   num_devices=NCORES)
    d_xT = nc.declare_dram_parameter("xT", [IN + 1, l_steps * BL], bf, isOutput=False)
    d_invm = nc.declare_dram_parameter("invm", [1, l_steps * BL], bf, isOutput=False)
    d_m63b = nc.declare_dram_parameter("m63b", [H, 2 * BL], f32, isOutput=False)
    d_wh = nc.declare_dram_parameter("wh", [H + 1, _WH_COLS], bf, isOutput=False)
    d_wx = nc.declare_dram_parameter("wx", [IN + 1, _WX_COLS], bf, isOutput=False)
    d_mrow = nc.declare_dram_parameter("mrow", [1, H], bf, isOutput=False)
    d_out = nc.declare_dram_parameter("out", [BL, t_steps * OUT], f32, isOutput=True)

    with tile.TileContext(nc) as tc, ExitStack() as ctx:
        const = ctx.enter_context(tc.tile_pool(name="const", bufs=1))
        temps = ctx.enter_context(tc.tile_pool(name="temps", bufs=3))
        p_r = ctx.enter_context(tc.tile_pool(name="p_r", bufs=1, space="PSUM"))
        p_z = ctx.enter_context(tc.tile_pool(name="p_z", bufs=1, space="PSUM"))
        p_a = ctx.enter_context(tc.tile_pool(name="p_a", bufs=1, space="PSUM"))
        p_b = ctx.enter_context(tc.tile_pool(name="p_b", bufs=1, space="PSUM"))
        p_ly = ctx.enter_context(tc.tile_pool(name="p_ly", bufs=2, space="PSUM"))
        p_y = ctx.enter_context(tc.tile_pool(name="p_y", bufs=2, space="PSUM"))

        xT = const.tile([IN + 1, l_steps * BL], bf)
        invm = const.tile([1, l_steps * BL], bf)
        m63b = const.tile([H, 2 * BL], f32)
        wh = const.tile([H + 1, _WH_COLS], bf)
        wx = const.tile([IN + 1, _WX_COLS], bf)
        mrow = const.tile([1, H], bf)
        # ping-pong recurrent state (row 64 = bias carrier)
        hd_pp = [const.tile([H + 1, BL], bf, tag=f"hd{i}", name=f"hd{i}") for i in (0, 1)]
        at_pp = [const.tile([H + 1, BL], bf, tag=f"at{i}", name=f"at{i}") for i in (0, 1)]
        ob_pp = [const.tile([H + 1, BL], bf, tag=f"ob{i}", name=f"ob{i}") for i in (0, 1)]
        ss_pp = [const.tile([H, BL], f32, tag=f"ss{i}", name=f"ss{i}")
                 for i in (0, 1)]
        sacc_a = const.tile([H, BL], f32)
        out_sb = const.tile([BL, t_steps * OUT], f32)

        for dst, src in ((xT, d_xT), (invm, d_invm), (m63b, d_m63b),
                         (wh, d_wh), (wx, d_wx), (mrow, d_mrow)):
            nc.sync.dma_start(out=dst, in_=src[:])

        for i in (0, 1):
            nc.vector.memset(hd_pp[i][H:H + 1, :], 1.0)
            nc.vector.memset(at_pp[i][H:H + 1, :], 0.0)
            nc.vector.memset(ob_pp[i][H:H + 1, :], 1.0)
        nc.vector.memset(hd_pp[0][0:H, :], 0.0)
        nc.vector.memset(ss_pp[0], 0.0)
        nc.vector.memset(ss_pp[1], 0.0)
        nc.vector.memset(sacc_a, 0.0)

        def mm(out, wcol, rhs, start, stop, wt=wh, width=H):
            nc.tensor.matmul(out[:], wt[:, wcol:wcol + width], rhs,
                             start=start, stop=stop)

        # ================= encoder =================
        pp = 0
        for t in range(l_steps):
            hdP, hdQ = hd_pp[pp], hd_pp[1 - pp]
            xs = xT[:, t * BL:(t + 1) * BL]
            pr = p_r.tile([H, BL], f32, tag="pr")
            pz = p_z.tile([H, BL], f32, tag="pz")
            pa = p_a.tile([H, BL], f32, tag="pa")
            pb = p_b.tile([H, BL], f32, tag="pb")
            last = t == l_steps - 1
            # x-side (prefetchable) first, then h-side; r-group closes first
            mm(pr, _XR, xs, True, False, wt=wx)
            mm(pz, _XZ, xs, True, False, wt=wx)
            if not last:
                nc.tensor.matmul(pz[:], mrow[:],
                                 invm[:, t * BL:(t + 1) * BL],
                                 start=False, stop=False)
            mm(pa, _XA, xs, True, True, wt=wx)
            mm(pr, _HR, hdP[:], False, True)
            mm(pz, _HZ, hdP[:], False, True)
            mm(pb, _HB, hdP[:], True, True)

            tr = temps.tile([H, BL], bf, tag="tr")
            nc.scalar.activation(out=tr, in_=pr[:], func=AF.Tanh)
            tz = temps.tile([H, BL], bf, tag="tz")
            nc.scalar.activation(out=tz, in_=pz[:], func=AF.Tanh)
            # unfolded: r*B = 0.5*(1+tr)*(2B') -> (tr+1) form
            t2 = temps.tile([H, BL], f32, tag="t2")
            nc.vector.scalar_tensor_tensor(out=t2, in0=tr, scalar=1.0,
                                           in1=pb[:], op0=OP.add, op1=OP.mult)
            nc.vector.tensor_add(out=pb[:], in0=t2, in1=pa[:])
            n = temps.tile([H, BL], bf, tag="n")
            nc.scalar.activation(out=n, in_=pb[:], func=AF.Tanh)
            if not last:
                tzp1 = temps.tile([H, BL], bf, tag="tzp1")
                nc.vector.tensor_scalar_add(out=tzp1, in0=tz, scalar1=1.0)
                tzh = temps.tile([H, BL], bf, tag="tzh")
                nc.vector.tensor_scalar(out=tzh, in0=tz, scalar1=-0.5,
                                        scalar2=0.5, op0=OP.mult, op1=OP.add)
                w2h = temps.tile([H, BL], bf, tag="w2h")
                nc.vector.tensor_mul(out=w2h, in0=tzh, in1=hdP[0:H, :])
                rr = temps.tile([H, BL], bf, tag="rr")
                nc.vector.tensor_mul(out=rr, in0=tzp1, in1=n)
                nc.vector.tensor_add(out=hdQ[0:H, :], in0=rr, in1=w2h)
            else:
                # d-form last step: unfrozen candidate + explicit m63 masking
                d = temps.tile([H, BL], f32, tag="d")
                nc.vector.scalar_tensor_tensor(out=d, in0=hdP[0:H, :],
                                               scalar=-0.5, in1=n,
                                               op0=OP.mult, op1=OP.add)
                tzd = temps.tile([H, BL], f32, tag="tzd")
                nc.vector.scalar_tensor_tensor(out=tzd, in0=tz, scalar=1.0,
                                               in1=d, op0=OP.add, op1=OP.mult)
                hn2 = temps.tile([H, BL], f32, tag="hn2")
                nc.vector.tensor_add(out=hn2, in0=tzd, in1=hdP[0:H, :])
                nc.vector.tensor_mul(out=ob_pp[1 - pp][0:H, :], in0=hn2,
                                     in1=m63b[:, BL:2 * BL])
                u = temps.tile([H, BL], f32, tag="u")
                nc.vector.tensor_mul(out=u, in0=tzd, in1=m63b[:, 0:BL])
                nc.vector.tensor_add(out=hdQ[0:H, :], in0=u, in1=hdP[0:H, :])
            pp = 1 - pp

        # ================= decoder =================
        # step t consumes hd(t-1), attn(t-1), oB(t-1); produces hd(t),
        # attn(t), oB(t); also finishes o(t-1)'s attention contribution
        # (logits/exp/accumulate) and emits y(t-1) into the y psum bank.
        ybank = None
        for t in range(t_steps):
            hdP, hdQ = hd_pp[pp], hd_pp[1 - pp]
            atP, atQ = at_pp[pp], at_pp[1 - pp]
            obP, obQ = ob_pp[pp], ob_pp[1 - pp]
            slot = (t - 1) % YGRP
            if t >= 1 and slot == 0:
                ybank = p_y.tile([BL, YGRP * OUT], f32, tag="ybank")

            # --- PE: attention tail mms for o(t-1), then gate mms ---
            ly = None
            if t >= 1:
                ly = p_ly.tile([H, BL], f32, tag="ly")
                mm(ly, _WA, obP[:], True, True)
                nc.tensor.matmul(ybank[:, OUT * slot:OUT * (slot + 1)],
                                 obP[:], wh[:, _WF:_WF + OUT],
                                 start=True, stop=True)
            pr = p_r.tile([H, BL], f32, tag="pr")
            pz = p_z.tile([H, BL], f32, tag="pz")
            pa = p_a.tile([H, BL], f32, tag="pa")
            pb = p_b.tile([H, BL], f32, tag="pb")
            if t == 0:
                # unfolded: h-side + input side through o(enc)
                mm(pr, _CR, obP[:], True, False)
                mm(pz, _CZ, obP[:], True, False)
                mm(pa, _CA, obP[:], True, True)
                mm(pr, _HR, hdP[:], False, True)
                mm(pz, _HZ, hdP[:], False, True)
            elif t == 1:
                # o(0) == h(0): fully folded, no attn side
                mm(pr, _FR, hdP[:], True, True)
                mm(pz, _FZ, hdP[:], True, True)
                mm(pa, _FA2, hdP[:], True, True)
            else:
                mm(pr, _CR, atP[:], True, False)
                mm(pz, _CZ, atP[:], True, False)
                mm(pa, _CA, atP[:], True, False)
                mm(pr, _FR, hdP[:], False, True)
                mm(pz, _FZ, hdP[:], False, True)
                mm(pa, _FA2, hdP[:], False, True)
            mm(pb, _HB, hdP[:], True, True)

            # --- ACT chain + exp ---
            tr = temps.tile([H, BL], bf, tag="tr")
            nc.scalar.activation(out=tr, in_=pr[:], func=AF.Tanh)
            e = None
            if t >= 1:
                e = temps.tile([H, BL], bf, tag="e")
                nc.scalar.activation(out=e, in_=ly[:], func=AF.Exp)
            tz = temps.tile([H, BL], bf, tag="tz")
            nc.scalar.activation(out=tz, in_=pz[:], func=AF.Tanh)

            # --- DVE chain --- (folded t>=1: tr*B'; unfolded t==0: (tr+1)*B')
            t2 = temps.tile([H, BL], f32, tag="t2")
            if t == 0:
                nc.vector.scalar_tensor_tensor(out=t2, in0=tr, scalar=1.0,
                                               in1=pb[:], op0=OP.add,
                                               op1=OP.mult)
            else:
                nc.vector.tensor_mul(out=t2, in0=tr, in1=pb[:])
            nc.vector.tensor_add(out=pb[:], in0=t2, in1=pa[:])
            n = temps.tile([H, BL], bf, tag="n")
            nc.scalar.activation(out=n, in_=pb[:], func=AF.Tanh)
            tzp1 = temps.tile([H, BL], bf, tag="tzp1")
            nc.vector.tensor_scalar_add(out=tzp1, in0=tz, scalar1=1.0)
            tzh = temps.tile([H, BL], bf, tag="tzh")
            nc.vector.tensor_scalar(out=tzh, in0=tz, scalar1=-0.5,
                                    scalar2=0.5, op0=OP.mult, op1=OP.add)
            w2h = temps.tile([H, BL], bf, tag="w2h")
            nc.vector.tensor_mul(out=w2h, in0=tzh, in1=hdP[0:H, :])
            rr = temps.tile([H, BL], bf, tag="rr")
            nc.vector.tensor_mul(out=rr, in0=tzp1, in1=n)
            nc.vector.tensor_add(out=hdQ[0:H, :], in0=rr, in1=w2h)

            # --- attention accumulate / normalize ---
            if t >= 1:
                eo = temps.tile([H, BL], bf, tag="eo")
                nc.gpsimd.tensor_mul(out=eo, in0=e, in1=obP[0:H, :])
                nc.gpsimd.tensor_add(out=sacc_a, in0=sacc_a, in1=eo)
                sP, sQ = ss_pp[pp], ss_pp[1 - pp]
                nc.gpsimd.tensor_add(out=sQ, in0=sP, in1=e)
                rec = temps.tile([H, BL], f32, tag="rec")
                nc.vector.reciprocal_approx_fast(out=rec, in_=sQ)
                nc.vector.tensor_mul(out=atQ[0:H, :], in0=sacc_a, in1=rec)
                nc.vector.scalar_tensor_tensor(out=obQ[0:H, :],
                                               in0=hdQ[0:H, :], scalar=0.5,
                                               in1=atQ[0:H, :],
                                               op0=OP.mult, op1=OP.add)
            else:
                nc.vector.tensor_scalar_mul(out=obQ[0:H, :],
                                            in0=hdQ[0:H, :], scalar1=0.5)

            # --- y bank evacuation every YGRP steps ---
            if t >= 1 and slot == YGRP - 1:
                g = (t - 1) // YGRP
                nc.scalar.copy(
                    out=out_sb[:, g * YGRP * OUT:(g + 1) * YGRP * OUT],
                    in_=ybank[:])
            pp = 1 - pp

        # final y(T-1) + last bank evacuation
        obP = ob_pp[pp]
        slot = (t_steps - 1) % YGRP
        if slot == 0:
            ybank = p_y.tile([BL, YGRP * OUT], f32, tag="ybank")
        nc.tensor.matmul(ybank[:, OUT * slot:OUT * (slot + 1)], obP[:],
                         wh[:, _WF:_WF + OUT], start=True, stop=True)
        g = (t_steps - 1) // YGRP
        nc.scalar.copy(out=out_sb[:, g * YGRP * OUT:(g + 1) * YGRP * OUT],
                       in_=ybank[:])

        nc.sync.dma_start(out=d_out[:], in_=out_sb)
    if compile:
        nc.compile()
    return nc


def _make_in_maps(inputs, l_steps=L, t_steps=T):
    x = np.asarray(inputs["x"], np.float32)
    lengths = np.asarray(inputs["lengths"])
    w = _prep_weights(inputs["Wih"], inputs["Whh"], inputs["bih"],
                      inputs["bhh"], inputs["Wf"], inputs["bf"],
                      inputs["Wa"], inputs["ba"])
    in_maps = []
    for c in range(NCORES):
        sl = slice(c * BL, (c + 1) * BL)
        xT, invm, m63b = _prep_core(x[sl], lengths[sl], l_steps)
        in_maps.append(dict(xT=xT, invm=invm, m63b=m63b, **w))
    return in_maps


def kernel(**inputs):
    global LAST_EXEC_NS, TRACE_DIR
    from concourse.bass_utils import run_bass_kernel_spmd
    t_steps = int(inputs.get("output_length", T))
    assert t_steps == T, f"hardcoded for output_length={T}, got {t_steps}"
    nc = build_nc()
    in_maps = _make_in_maps(inputs)
    kw = {}
    if TRACE:
        import tempfile
        TRACE_DIR = tempfile.mkdtemp(prefix="bass_trace_")
        kw = dict(trace=True, tmpdir=TRACE_DIR)
    res = None
    for attempt in range(3):
        try:
            res = run_bass_kernel_spmd(nc, in_maps, list(range(NCORES)), **kw)
            break
        except Exception:
            # transient device errors (e.g. NRT_EXEC_UNIT_UNRECOVERABLE) have
            # been observed under axon; the identical NEFF passes on retry
            if attempt == 2:
                raise
    LAST_EXEC_NS = res.exec_time_ns
    outs = [np.asarray(res.results[c]["out"]).reshape(BL, T, OUT)
            for c in range(NCORES)]
    return np.concatenate(outs, axis=0)


# revision 10
# speedup vs baseline: 1.1315x; 1.1315x over previous
"""Trainium2 Bass kernel for nn_AutoregressiveGRUWithAttention.

Strategy (data-parallel over batch, 8 cores x 128 batch):
  Feature-on-partition layout, bf16 state, tanh-only gates
  (sigmoid(v)=0.5+0.5tanh(v/2) folded into weights; z negated).

  v2 changes vs baseline:
  - Doubled hidden state hd = 2h (h-rows of all h-consuming weight blocks
    pre-scaled by 0.5). GRU mix tail becomes:
      rr  = (1+tz) * n            (DVE, on chain)
      w2  = (tz-1) * hd_prev      (GPSIMD, off chain)
      hd' = rr - 0.5*w2           (DVE)  == 2*((1-z)n + z h)
    replacing the baseline's p1/pp/rr/h' 4-op tail.
  - y outputs accumulate in dedicated PSUM banks (32 steps per bank,
    ping-pong) and are evacuated by one ScalarE copy per 32 steps instead
    of a per-step scalar.copy.
  - Attention tail is its own software pipeline: e=exp(logits) bf16,
    eo=e*o (GPSIMD bf16), separate s/acc accumulators so rec=1/s does not
    wait for the eo add; attn = acc*rec (DVE).
  - State tiles (hd / attn / oB) are manually ping-ponged to remove
    write-after-read hazards from the recurrence chain.

PSUM banks: pr, pz, pa, pb (bufs=1 each), ly logits (bufs=2),
            y-accum (bufs=2, [128, 32*13]).
"""
import numpy as np
import ml_dtypes

B, L, T, IN, H, OUT = 1024, 64, 128, 13, 64, 13
NCORES, BL = 8, 128
BIG = 60.0
BF16 = ml_dtypes.bfloat16
YGRP = 32                      # y slots per psum bank

# wh pack column offsets [65 x 653] (bf16)
_HR, _HZ, _HB, _CR, _CZ, _CA, _WA = 0, 64, 128, 192, 256, 320, 384
_FR, _FZ, _FA2, _WF = 448, 512, 576, 640
_WH_COLS = 653
# wx pack column offsets [14 x 192] (bf16)
_XR, _XZ, _XA = 0, 64, 128
_WX_COLS = 192

LAST_EXEC_NS = None
TRACE = False
TRACE_DIR = None
WARM_DUMMIES = 0


def _prep_weights(Wih, Whh, bih, bhh, Wf, bf, Wa, ba):
    f8 = np.float64
    Wih, Whh, bih, bhh, Wf, bf, Wa, ba = [np.asarray(a, f8) for a in
                                          (Wih, Whh, bih, bhh, Wf, bf, Wa, ba)]
    Wr, Wz, Wn = Wih[0:H], Wih[H:2 * H], Wih[2 * H:3 * H]
    Ur, Uz, Un = Whh[0:H], Whh[H:2 * H], Whh[2 * H:3 * H]
    br_i, bz_i, bn_i = bih[0:H], bih[H:2 * H], bih[2 * H:3 * H]
    br_h, bz_h, bn_h = bhh[0:H], bhh[H:2 * H], bhh[2 * H:3 * H]

    def blk(rows, rowbias, scale):
        m = np.zeros((H + 1, rows.shape[0]), f8)
        m[0:H] = scale * rows.T
        m[H] = scale * rowbias
        return m

    def half(m):
        # doubled-h: blocks that multiply hd get their h-rows halved
        m2 = m.copy()
        m2[0:H] *= 0.5
        return m2

    HRm = blk(Ur, br_i + br_h, 0.5)
    HZm = blk(Uz, bz_i + bz_h, -0.5)
    HBm = blk(Un, bn_h, 0.5)                        # B' = 0.5*(nh + bhh_n)
    CRm = blk((Wf.T @ Wr.T).T, bf @ Wr.T, 0.5)
    CZm = blk((Wf.T @ Wz.T).T, bf @ Wz.T, -0.5)
    CAm = blk((Wf.T @ Wn.T).T, bf @ Wn.T + bn_i, 1.0)
    WAm = blk(Wa, ba, 1.0)

    wh = np.zeros((H + 1, _WH_COLS), f8)
    for col, m in ((_HR, half(HRm)), (_HZ, half(HZm)), (_HB, half(HBm)),
                   (_CR, CRm), (_CZ, CZm), (_CA, CAm), (_WA, WAm),
                   (_FR, half(HRm + CRm)), (_FZ, half(HZm + CZm)),
                   (_FA2, half(CAm + HBm))):
        wh[:, col:col + H] = m
    wh[0:H, _WF:_WF + OUT] = Wf.T
    wh[H, _WF:_WF + OUT] = bf

    wx = np.zeros((IN + 1, _WX_COLS), f8)
    wx[0:IN, _XR:_XR + H] = 0.5 * Wr.T
    wx[0:IN, _XZ:_XZ + H] = -0.5 * Wz.T
    wx[0:IN, _XA:_XA + H] = Wn.T
    wx[IN, _XA:_XA + H] = bn_i

    mrow = np.full((1, H), -0.5 * BIG, f8)
    return dict(
        wh=np.ascontiguousarray(wh, BF16),
        wx=np.ascontiguousarray(wx, BF16),
        mrow=np.ascontiguousarray(mrow, BF16),
    )


def _prep_core(x_core, len_core, l_steps=L):
    x_core = np.asarray(x_core, np.float32)
    xT = np.zeros((IN + 1, l_steps, BL), np.float32)
    xT[0:IN] = np.transpose(x_core[:, 0:l_steps, :], (2, 1, 0))
    xT[IN] = 1.0
    valid = (np.arange(l_steps)[:, None] < np.asarray(len_core)[None, :])
    invm = (1.0 - valid.astype(np.float32)).reshape(1, l_steps * BL)
    m63 = valid[l_steps - 1].astype(np.float32)
    m63bc = np.broadcast_to(m63, (H, BL)).astype(np.float32)
    m63b = np.concatenate([m63bc, 0.5 * m63bc], axis=1)   # [H, 2*BL]
    return (np.ascontiguousarray(xT.reshape(IN + 1, l_steps * BL), BF16),
            np.ascontiguousarray(invm, BF16),
            np.ascontiguousarray(m63b, np.float32))


def build_nc(l_steps=L, t_steps=T, compile=True, WARM=None):
    if WARM is None:
        WARM = WARM_DUMMIES
    import concourse.bacc as bacc
    import concourse.tile as tile
    from concourse import mybir
    from contextlib import ExitStack

    f32 = mybir.dt.float32
    bf = mybir.dt.bfloat16
    AF = mybir.ActivationFunctionType
    OP = mybir.AluOpType

    assert t_steps % YGRP == 0
    n_ygrp = t_steps // YGRP

    nc = bacc.Bacc("TRN2", target_bir_lowering=False, debug=False,
                   num_devices=NCORES)
    d_xT = nc.declare_dram_parameter("xT", [IN + 1, l_steps * BL], bf, isOutput=False)
    d_invm = nc.declare_dram_parameter("invm", [1, l_steps * BL], bf, isOutput=False)
    d_m63b = nc.declare_dram_parameter("m63b", [H, 2 * BL], f32, isOutput=False)
    d_wh = nc.declare_dram_parameter("wh", [H + 1, _WH_COLS], bf, isOutput=False)
    d_wx = nc.declare_dram_parameter("wx", [IN + 1, _WX_COLS], bf, isOutput=False)
    d_mrow = nc.declare_dram_parameter("mrow", [1, H], bf, isOutput=False)
    d_out = nc.declare_dram_parameter("out", [BL, t_steps * OUT], f32, isOutput=True)

    with tile.TileContext(nc) as tc, ExitStack() as ctx:
        const = ctx.enter_context(tc.tile_pool(name="const", bufs=1))
        temps = ctx.enter_context(tc.tile_pool(name="temps", bufs=3))
        p_r = ctx.enter_context(tc.tile_pool(name="p_r", bufs=1, space="PSUM"))
        p_z = ctx.enter_context(tc.tile_pool(name="p_z", bufs=1, space="PSUM"))
        p_a = ctx.enter_context(tc.tile_pool(name="p_a", bufs=1, space="PSUM"))
        p_b = ctx.enter_context(tc.tile_pool(name="p_b", bufs=1, space="PSUM"))
        p_ly = ctx.enter_context(tc.tile_pool(name="p_ly", bufs=2, space="PSUM"))
        p_y = ctx.enter_context(tc.tile_pool(name="p_y", bufs=2, space="PSUM"))

        xT = const.tile([IN + 1, l_steps * BL], bf)
        invm = const.tile([1, l_steps * BL], bf)
        m63b = const.tile([H, 2 * BL], f32)
        wh = const.tile([H + 1, _WH_COLS], bf)
        wx = const.tile([IN + 1, _WX_COLS], bf)
        mrow = const.tile([1, H], bf)
        # ping-pong recurrent state (row 64 = bias carrier)
        hd_pp = [const.tile([H + 1, BL], bf, tag=f"hd{i}", name=f"hd{i}") for i in (0, 1)]
        at_pp = [const.tile([H + 1, BL], bf, tag=f"at{i}", name=f"at{i}") for i in (0, 1)]
        ob_pp = [const.tile([H + 1, BL], bf, tag=f"ob{i}", name=f"ob{i}") for i in (0, 1)]
        ss_pp = [const.tile([H, BL], f32, tag=f"ss{i}", name=f"ss{i}")
                 for i in (0, 1)]
        sacc_a = const.tile([H, BL], f32)
        out_sb = const.tile([BL, t_steps * OUT], f32)

        for dst, src in ((xT, d_xT), (invm, d_invm), (m63b, d_m63b),
                         (wh, d_wh), (wx, d_wx), (mrow, d_mrow)):
            nc.sync.dma_start(out=dst, in_=src[:])

        for i in (0, 1):
            nc.vector.memset(hd_pp[i][H:H + 1, :], 1.0)
            nc.vector.memset(at_pp[i][H:H + 1, :], 0.0)
            nc.vector.memset(ob_pp[i][H:H + 1, :], 1.0)
        nc.vector.memset(hd_pp[0][0:H, :], 0.0)
        nc.vector.memset(ss_pp[0], 0.0)
        nc.vector.memset(ss_pp[1], 0.0)
        nc.vector.memset(sacc_a, 0.0)

        def mm(out, wcol, rhs, start, stop, wt=wh, width=H):
            nc.tensor.matmul(out[:], wt[:, wcol:wcol + width], rhs,
                             start=start, stop=stop)

        # HAM warm-up: ~6us of back-to-back matmuls into a scratch psum bank
        # so the PE clock-gate opens (1.2 -> 2.4 GHz) before the recurrence.
        warmb = p_y.tile([BL, YGRP * OUT], f32, tag="ybank", name="warmb")
        for _ in range(16):
            nc.tensor.matmul(warmb[0:H, :], wx[:, 0:H],
                             xT[:, 0:YGRP * OUT], start=True, stop=True)

        # ================= encoder =================
        pp = 0
        for t in range(l_steps):
            hdP, hdQ = hd_pp[pp], hd_pp[1 - pp]
            xs = xT[:, t * BL:(t + 1) * BL]
            pr = p_r.tile([H, BL], f32, tag="pr")
            pz = p_z.tile([H, BL], f32, tag="pz")
            pa = p_a.tile([H, BL], f32, tag="pa")
            pb = p_b.tile([H, BL], f32, tag="pb")
            last = t == l_steps - 1
            # x-side (prefetchable) first, then h-side; r-group closes first
            mm(pr, _XR, xs, True, False, wt=wx)
            mm(pz, _XZ, xs, True, False, wt=wx)
            if not last:
                nc.tensor.matmul(pz[:], mrow[:],
                                 invm[:, t * BL:(t + 1) * BL],
                                 start=False, stop=False)
            mm(pa, _XA, xs, True, True, wt=wx)
            mm(pr, _HR, hdP[:], False, True)
            mm(pz, _HZ, hdP[:], False, True)
            mm(pb, _HB, hdP[:], True, True)

            tr = temps.tile([H, BL], bf, tag="tr")
            nc.scalar.activation(out=tr, in_=pr[:], func=AF.Tanh)
            tz = temps.tile([H, BL], bf, tag="tz")
            nc.scalar.activation(out=tz, in_=pz[:], func=AF.Tanh)
            # unfolded: r*B = 0.5*(1+tr)*(2B') -> (tr+1) form
            t2 = temps.tile([H, BL], f32, tag="t2")
            nc.vector.scalar_tensor_tensor(out=t2, in0=tr, scalar=1.0,
                                           in1=pb[:], op0=OP.add, op1=OP.mult)
            nc.vector.tensor_add(out=pb[:], in0=t2, in1=pa[:])
            n = temps.tile([H, BL], bf, tag="n")
            nc.scalar.activation(out=n, in_=pb[:], func=AF.Tanh)
            if not last:
                rr = temps.tile([H, BL], bf, tag="rr")
                nc.vector.scalar_tensor_tensor(out=rr, in0=tz, scalar=1.0,
                                               in1=n, op0=OP.add, op1=OP.mult)
                w2 = temps.tile([H, BL], bf, tag="w2")
                nc.vector.scalar_tensor_tensor(out=w2, in0=tz, scalar=1.0,
                                               in1=hdP[0:H, :],
                                               op0=OP.subtract, op1=OP.mult)
                nc.vector.scalar_tensor_tensor(out=hdQ[0:H, :], in0=w2,
                                               scalar=-0.5, in1=rr,
                                               op0=OP.mult, op1=OP.add)
            else:
                # d-form last step: unfrozen candidate + explicit m63 masking
                d = temps.tile([H, BL], f32, tag="d")
                nc.vector.scalar_tensor_tensor(out=d, in0=hdP[0:H, :],
                                               scalar=-0.5, in1=n,
                                               op0=OP.mult, op1=OP.add)
                tzd = temps.tile([H, BL], f32, tag="tzd")
                nc.vector.scalar_tensor_tensor(out=tzd, in0=tz, scalar=1.0,
                                               in1=d, op0=OP.add, op1=OP.mult)
                hn2 = temps.tile([H, BL], f32, tag="hn2")
                nc.vector.tensor_add(out=hn2, in0=tzd, in1=hdP[0:H, :])
                nc.vector.tensor_mul(out=ob_pp[1 - pp][0:H, :], in0=hn2,
                                     in1=m63b[:, BL:2 * BL])
                u = temps.tile([H, BL], f32, tag="u")
                nc.vector.tensor_mul(out=u, in0=tzd, in1=m63b[:, 0:BL])
                nc.vector.tensor_add(out=hdQ[0:H, :], in0=u, in1=hdP[0:H, :])
            pp = 1 - pp

        # ================= decoder =================
        # step t consumes hd(t-1), attn(t-1), oB(t-1); produces hd(t),
        # attn(t), oB(t); also finishes o(t-1)'s attention contribution
        # (logits/exp/accumulate) and emits y(t-1) into the y psum bank.
        ybank = None
        for t in range(t_steps):
            hdP, hdQ = hd_pp[pp], hd_pp[1 - pp]
            atP, atQ = at_pp[pp], at_pp[1 - pp]
            obP, obQ = ob_pp[pp], ob_pp[1 - pp]
            slot = (t - 1) % YGRP
            if t >= 1 and slot == 0:
                ybank = p_y.tile([BL, YGRP * OUT], f32, tag="ybank")

            # --- PE: attention tail mms for o(t-1), then gate mms ---
            ly = None
            pr = p_r.tile([H, BL], f32, tag="pr")
            pz = p_z.tile([H, BL], f32, tag="pz")
            pa = p_a.tile([H, BL], f32, tag="pa")
            pb = p_b.tile([H, BL], f32, tag="pb")
            if t == 0:
                mm(pr, _CR, obP[:], True, False)
                mm(pr, _HR, hdP[:], False, True)
                mm(pz, _CZ, obP[:], True, False)
                mm(pz, _HZ, hdP[:], False, True)
                mm(pa, _CA, obP[:], True, True)
            elif t == 1:
                ly = p_ly.tile([H, BL], f32, tag="ly")
                mm(pr, _FR, hdP[:], True, True)
                mm(ly, _WA, obP[:], True, True)
                mm(pz, _FZ, hdP[:], True, True)
                nc.tensor.matmul(ybank[:, OUT * slot:OUT * (slot + 1)],
                                 obP[:], wh[:, _WF:_WF + OUT],
                                 start=True, stop=True)
                mm(pa, _FA2, hdP[:], True, True)
            else:
                ly = p_ly.tile([H, BL], f32, tag="ly")
                mm(pr, _CR, atP[:], True, False)
                mm(pr, _FR, hdP[:], False, True)
                mm(ly, _WA, obP[:], True, True)
                mm(pz, _CZ, atP[:], True, False)
                mm(pz, _FZ, hdP[:], False, True)
                nc.tensor.matmul(ybank[:, OUT * slot:OUT * (slot + 1)],
                                 obP[:], wh[:, _WF:_WF + OUT],
                                 start=True, stop=True)
                mm(pa, _CA, atP[:], True, False)
                mm(pa, _FA2, hdP[:], False, True)
            mm(pb, _HB, hdP[:], True, True)

            # --- ACT chain + exp ---
            tr = temps.tile([H, BL], bf, tag="tr")
            nc.scalar.activation(out=tr, in_=pr[:], func=AF.Tanh)
            e = None
            if t >= 1:
                e = temps.tile([H, BL], bf, tag="e")
                nc.scalar.activation(out=e, in_=ly[:], func=AF.Exp)
            tz = temps.tile([H, BL], bf, tag="tz")
            nc.scalar.activation(out=tz, in_=pz[:], func=AF.Tanh)

            # --- DVE chain --- (folded t>=1: tr*B'; unfolded t==0: (tr+1)*B')
            t2 = temps.tile([H, BL], f32, tag="t2")
            if t == 0:
                nc.vector.scalar_tensor_tensor(out=t2, in0=tr, scalar=1.0,
                                               in1=pb[:], op0=OP.add,
                                               op1=OP.mult)
            else:
                nc.vector.tensor_mul(out=t2, in0=tr, in1=pb[:])
            nc.vector.tensor_add(out=pb[:], in0=t2, in1=pa[:])
            n = temps.tile([H, BL], bf, tag="n")
            nc.scalar.activation(out=n, in_=pb[:], func=AF.Tanh)
            rr = temps.tile([H, BL], bf, tag="rr")
            nc.vector.scalar_tensor_tensor(out=rr, in0=tz, scalar=1.0,
                                           in1=n, op0=OP.add, op1=OP.mult)
            w2 = temps.tile([H, BL], bf, tag="w2")
            nc.vector.scalar_tensor_tensor(out=w2, in0=tz, scalar=1.0,
                                           in1=hdP[0:H, :],
                                           op0=OP.subtract, op1=OP.mult)
            nc.vector.scalar_tensor_tensor(out=hdQ[0:H, :], in0=w2,
                                           scalar=-0.5, in1=rr,
                                           op0=OP.mult, op1=OP.add)

            # --- attention accumulate / normalize ---
            if t >= 1:
                eo = temps.tile([H, BL], bf, tag="eo")
                nc.gpsimd.tensor_mul(out=eo, in0=e, in1=obP[0:H, :])
                nc.gpsimd.tensor_add(out=sacc_a, in0=sacc_a, in1=eo)
                sP, sQ = ss_pp[pp], ss_pp[1 - pp]
                nc.gpsimd.tensor_add(out=sQ, in0=sP, in1=e)
                rec = temps.tile([H, BL], f32, tag="rec")
                nc.vector.reciprocal_approx_fast(out=rec, in_=sQ)
                nc.vector.tensor_mul(out=atQ[0:H, :], in0=sacc_a, in1=rec)
                nc.vector.scalar_tensor_tensor(out=obQ[0:H, :],
                                               in0=hdQ[0:H, :], scalar=0.5,
                                               in1=atQ[0:H, :],
                                               op0=OP.mult, op1=OP.add)
            else:
                nc.vector.tensor_scalar_mul(out=obQ[0:H, :],
                                            in0=hdQ[0:H, :], scalar1=0.5)

            # --- y bank evacuation every YGRP steps ---
            if t >= 1 and slot == YGRP - 1:
                g = (t - 1) // YGRP
                nc.scalar.copy(
                    out=out_sb[:, g * YGRP * OUT:(g + 1) * YGRP * OUT],
                    in_=ybank[:])
            pp = 1 - pp

        # final y(T-1) + last bank evacuation
        obP = ob_pp[pp]
        slot = (t_steps - 1) % YGRP
        if slot == 0:
            ybank = p_y.tile([BL, YGRP * OUT], f32, tag="ybank")
        nc.tensor.matmul(ybank[:, OUT * slot:OUT * (slot + 1)], obP[:],
                         wh[:, _WF:_WF + OUT], start=True, stop=True)
        g = (t_steps - 1) // YGRP
        nc.scalar.copy(out=out_sb[:, g * YGRP * OUT:(g + 1) * YGRP * OUT],
                       in_=ybank[:])

        nc.sync.dma_start(out=d_out[:], in_=out_sb)
    if compile:
        nc.compile()
    return nc


def _make_in_maps(inputs, l_steps=L, t_steps=T):
    x = np.asarray(inputs["x"], np.float32)
    lengths = np.asarray(inputs["lengths"])
    w = _prep_weights(inputs["Wih"], inputs["Whh"], inputs["bih"],
                      inputs["bhh"], inputs["Wf"], inputs["bf"],
                      inputs["Wa"], inputs["ba"])
    in_maps = []
    for c in range(NCORES):
        sl = slice(c * BL, (c + 1) * BL)
        xT, invm, m63b = _prep_core(x[sl], lengths[sl], l_steps)
        in_maps.append(dict(xT=xT, invm=invm, m63b=m63b, **w))
    return in_maps


def kernel(**inputs):
    global LAST_EXEC_NS, TRACE_DIR
    from concourse.bass_utils import run_bass_kernel_spmd
    t_steps = int(inputs.get("output_length", T))
    assert t_steps == T, f"hardcoded for output_length={T}, got {t_steps}"
    nc = build_nc()
    in_maps = _make_in_maps(inputs)
    kw = {}
    if TRACE:
        import tempfile
        TRACE_DIR = tempfile.mkdtemp(prefix="bass_trace_")
        kw = dict(trace=True, tmpdir=TRACE_DIR)
    res = None
    for attempt in range(3):
        try:
            res = run_bass_kernel_spmd(nc, in_maps, list(range(NCORES)), **kw)
            break
        except Exception:
            # transient device errors (e.g. NRT_EXEC_UNIT_UNRECOVERABLE) have
            # been observed under axon; the identical NEFF passes on retry
            if attempt == 2:
                raise
    LAST_EXEC_NS = res.exec_time_ns
    outs = [np.asarray(res.results[c]["out"]).reshape(BL, T, OUT)
            for c in range(NCORES)]
    return np.concatenate(outs, axis=0)
